# revision 1
# baseline (speedup 1.0000x reference)
"""Causal self-attention (B=2, T=2048, D=1024, H=16) on 8 trn2 cores.

Sharding: tensor-parallel over heads x data-parallel over batch.
Core c handles batch b = c // 4, head group g = c % 4 (heads 4g..4g+3).
Host pre-slices/pre-transposes weight+activation shards; each core
returns a partial y (its heads' contribution); host sums groups of 4.
"""

import os
import sys

for _p in ("/opt/trn_rl_repo", "/root/.axon_site/_ro/trn_rl_repo"):
    if os.path.isdir(_p) and _p not in sys.path:
        sys.path.insert(0, _p)

import numpy as np

import concourse.bass as bass
import concourse.mybir as mybir
import concourse.tile as tile
from concourse import bacc
from concourse.bass_utils import run_bass_kernel_spmd

F32 = mybir.dt.float32
F32R = mybir.dt.float32r

B, T, C = 2, 2048, 1024
NHEAD_TOT = 16
DH = 64
NCORES = 8
NH = 4          # heads per core
NPAIR = 2       # head pairs per core
CK = C // 128   # contraction chunks (8)
TT = 512        # attention t-tile width
NTT = T // TT   # 4
NSCH = T // 128  # s chunks (16)
FQK = 2 * NH * DH  # 512 cols of qkv^T for q+k
FV = NH * DH       # 256 cols for v


def r32(ap):
    return ap.bitcast(F32R)


def build_nc(dbg=False):
    nc = bacc.Bacc("TRN2", target_bir_lowering=False, debug=False)

    xT = nc.dram_tensor("xT", [C, T], F32R, kind="ExternalInput")
    wqkvT = nc.dram_tensor("wqkvT", [C, FQK + FV], F32R, kind="ExternalInput")
    woutT = nc.dram_tensor("woutT", [NH * DH, C], F32R, kind="ExternalInput")
    y = nc.dram_tensor("y", [T, C], F32, kind="ExternalOutput")
    L_dram = nc.dram_tensor("L_scratch", [2 * NPAIR * NTT, TT], F32)
    R_dram = nc.dram_tensor("R_scratch", [2 * NPAIR * NTT, TT], F32R)
    if dbg:
        dbg_qkT = nc.dram_tensor("dbg_qkT", [128, 4, T], F32R, kind="ExternalOutput")
        dbg_v = nc.dram_tensor("dbg_v", [128, NSCH, NH, DH + 1], F32R, kind="ExternalOutput")
        dbg_oT = nc.dram_tensor("dbg_oT", [128, NPAIR, T], F32R, kind="ExternalOutput")
        dbg_pt = nc.dram_tensor("dbg_pt", [128, 2, TT], F32R, kind="ExternalOutput")
        dbg_pv = nc.dram_tensor("dbg_pv", [DH + 1, TT], F32, kind="ExternalOutput")
        dbg_bc = nc.dram_tensor("dbg_bc", [128, TT], F32, kind="ExternalOutput")

    EXP = mybir.ActivationFunctionType.Exp

    with tile.TileContext(nc) as tc:
        with (
            tc.tile_pool(name="const", bufs=1) as const,
            tc.tile_pool(name="ptp", bufs=4) as ptp,
            tc.tile_pool(name="bcp", bufs=2) as bcp,
            tc.tile_pool(name="rcp", bufs=2) as rcp,
            tc.tile_pool(name="yp", bufs=2) as yp,
            tc.tile_pool(name="psA", bufs=2, space="PSUM") as psA,
            tc.tile_pool(name="psV", bufs=4, space="PSUM") as psV,
        ):
            # ---- persistent SBUF ----
            xT_sb = const.tile([128, CK, T], F32R)          # x^T  (c-major)
            wqkvT_sb = const.tile([128, CK, FQK + FV], F32R)  # W_qkv^T cols [q(4x64)|k(4x64)|v(4x64)]
            woutT_sb = const.tile([128, NPAIR, C], F32R)    # W_out^T rows per head pair
            qkT_sb = const.tile([128, 4, T], F32R)          # [qPair0|qPair1|kPair0|kPair1] x T
            v_sb = const.tile([128, NSCH, NH, DH + 1], F32R)  # V (s-major) + ones column
            oT_sb = const.tile([128, NPAIR, T], F32R)       # normalized O^T, pair-stacked

            for ci in range(CK):
                nc.sync.dma_start(xT_sb[:, ci, :], xT[ci * 128:(ci + 1) * 128, :])
                nc.sync.dma_start(wqkvT_sb[:, ci, :], wqkvT[ci * 128:(ci + 1) * 128, :])
            for pr in range(NPAIR):
                nc.sync.dma_start(woutT_sb[:, pr, :], woutT[pr * 128:(pr + 1) * 128, :])
            # 1.0f bit pattern; direct f32r memset is rejected by walrus codegen
            nc.vector.memset(v_sb[:, :, :, DH:DH + 1].bitcast(mybir.dt.uint32),
                             0x3F800000)
            ones1 = const.tile([1, 64], F32R)  # lhsT for recipL row->partition broadcast
            nc.vector.memset(ones1.bitcast(mybir.dt.uint32), 0x3F800000)

            # ---- QKV projection ----
            # q^T/k^T: psum[f128, t512] = sum_c wqkvT[c, f].T @ xT[c, t]
            for ft in range(4):
                for tt in range(NTT):
                    ps = psA.tile([128, 2, TT], F32)
                    for ci in range(CK):
                        nc.tensor.matmul(
                            ps[:, 0, :],
                            wqkvT_sb[:, ci, ft * 128:(ft + 1) * 128],
                            xT_sb[:, ci, tt * TT:(tt + 1) * TT],
                            start=(ci == 0), stop=(ci == CK - 1),
                        )
                    nc.vector.tensor_copy(qkT_sb[:, ft, tt * TT:(tt + 1) * TT], ps[:, 0, :])
            # v natural: psum[t128, f256] = xT[c, t].T @ wqkvT[c, v]
            for si in range(NSCH):
                ps = psA.tile([128, 2, TT], F32)
                for ci in range(CK):
                    nc.tensor.matmul(
                        ps[:, 0, 0:FV],
                        xT_sb[:, ci, si * 128:(si + 1) * 128],
                        wqkvT_sb[:, ci, FQK:FQK + FV],
                        start=(ci == 0), stop=(ci == CK - 1),
                    )
                nc.vector.tensor_copy(
                    v_sb[:, si, :, 0:DH],
                    ps[:, 0, 0:FV].rearrange("p (h d) -> p h d", h=NH),
                )

            # ---- attention (S^T orientation), per head pair ----
            # oT_sb holds UNNORMALIZED O^T during the loop; softmax sums (L)
            # are shipped to DRAM and normalization happens in a batched end
            # phase so PV psums are released by plain copies (no recip chain).
            for pr in range(NPAIR):
                for tt in range(NTT):
                    n_ss = 4 * (tt + 1)  # causal: s-chunks 0 .. 4*tt+3
                    pv = [psV.tile([DH + 1, TT], F32, tag="pv", name=f"pv{pr}_{tt}_{k}")
                          for k in range(2)]
                    for sq in range(n_ss // 2):
                        for hi in range(2):
                            h = pr * 2 + hi
                            ps = psA.tile([128, 2, TT], F32)
                            for i in range(2):
                                ss = 2 * sq + i
                                nc.tensor.matmul(
                                    ps[:, i, :],
                                    qkT_sb[hi * 64:(hi + 1) * 64, 2 + pr, ss * 128:(ss + 1) * 128],
                                    qkT_sb[hi * 64:(hi + 1) * 64, pr, tt * TT:(tt + 1) * TT],
                                )
                            pt = ptp.tile([128, 2, TT], F32R)
                            nc.scalar.activation(pt, ps, EXP, scale=0.125)
                            if 2 * sq >= 4 * tt:  # diagonal quad: zero where s > t
                                nc.gpsimd.affine_select(
                                    out=pt, in_=pt,
                                    compare_op=mybir.AluOpType.is_ge,
                                    fill=0.0,
                                    base=tt * TT - 2 * sq * 128,
                                    channel_multiplier=-1,
                                    pattern=[[-128, 2], [1, TT]],
                                )
                            if dbg and pr == 0 and tt == 0 and sq == 0 and hi == 0:
                                nc.sync.dma_start(dbg_pt[:], pt)
                            for i in range(2):
                                ss = 2 * sq + i
                                nc.tensor.matmul(
                                    pv[hi],
                                    v_sb[:, ss, h, :],
                                    pt[:, i, :],
                                    start=(ss == 0), stop=(ss == n_ss - 1),
                                )
                    if dbg and pr == 0 and tt == 0:
                        pvcpy = bcp.tile([DH + 1, TT], F32, tag="pvcpy")
                        nc.vector.tensor_copy(pvcpy, pv[0])
                        nc.sync.dma_start(dbg_pv[:], pvcpy)
                    for hi in range(2):
                        idx = (pr * NTT + tt) * 2 + hi
                        nc.vector.tensor_copy(
                            oT_sb[hi * 64:(hi + 1) * 64, pr, tt * TT:(tt + 1) * TT],
                            pv[hi][0:DH, :],
                        )
                        lrow = rcp.tile([1, TT], F32, tag="lrow", name=f"lrow{idx}")
                        nc.vector.tensor_copy(lrow, pv[hi][DH:DH + 1, :])
                        nc.sync.dma_start(L_dram[idx:idx + 1, :], lrow[0:1, :])

            # ---- batched softmax normalization ----
            # gather all 16 L rows as [128, 64], one fast reciprocal, ship
            # back, then per-tile outer-product broadcast + multiply.
            lsq = bcp.tile([128, 64], F32, tag="lsq")
            nc.sync.dma_start(lsq, L_dram[:, :].rearrange("r (s j) -> (r s) j", j=64))
            with nc.allow_low_precision("f32r recip feeds f32r matmul rhs"):
                rsq = bcp.tile([128, 64], F32R, tag="rsq")
                nc.vector.reciprocal(rsq, lsq)
            nc.sync.dma_start(R_dram[:, :].rearrange("r (s j) -> (r s) j", j=64), rsq)
            for pr in range(NPAIR):
                for tt in range(NTT):
                    bq = psA.tile([128, 2, TT], F32, tag="ps", name=f"bq{pr}_{tt}")
                    bc = bcp.tile([128, TT], F32)
                    for hi in range(2):
                        idx = (pr * NTT + tt) * 2 + hi
                        rcr = rcp.tile([1, TT], F32R, tag="rcr", name=f"rcr{idx}")
                        nc.sync.dma_start(rcr[0:1, :], R_dram[idx:idx + 1, :])
                        nc.tensor.matmul(bq[:, hi, :][0:64, :], ones1, rcr)
                        nc.vector.tensor_copy(
                            bc[hi * 64:(hi + 1) * 64, :], bq[:, hi, :][0:64, :]
                        )
                        nc.vector.tensor_mul(
                            oT_sb[hi * 64:(hi + 1) * 64, pr, tt * TT:(tt + 1) * TT],
                            oT_sb[hi * 64:(hi + 1) * 64, pr, tt * TT:(tt + 1) * TT],
                            bc[hi * 64:(hi + 1) * 64, :],
                        )
                    if dbg and pr == 0 and tt == 0:
                        nc.sync.dma_start(dbg_bc[:], bc)

            if dbg:
                nc.sync.dma_start(dbg_qkT[:], qkT_sb)
                nc.sync.dma_start(dbg_v[:], v_sb)
                nc.sync.dma_start(dbg_oT[:], oT_sb)

            # ---- output projection: y[t, o] = sum_pr oT[d, t].T @ woutT[d, o] ----
            for tq in range(T // 128):
                for ot in range(C // TT):
                    ps = psA.tile([128, 2, TT], F32)
                    for pr in range(NPAIR):
                        nc.tensor.matmul(
                            ps[:, 0, :],
                            oT_sb[:, pr, tq * 128:(tq + 1) * 128],
                            woutT_sb[:, pr, ot * TT:(ot + 1) * TT],
                            start=(pr == 0), stop=(pr == NPAIR - 1),
                        )
                    yt = yp.tile([128, TT], F32)
                    nc.vector.tensor_copy(yt, ps[:, 0, :])
                    nc.sync.dma_start(y[tq * 128:(tq + 1) * 128, ot * TT:(ot + 1) * TT], yt)

    nc.compile()
    return nc


_NC_CACHE = None


def _get_nc():
    global _NC_CACHE
    if _NC_CACHE is None:
        _NC_CACHE = build_nc()
    return _NC_CACHE


def make_in_maps(x, W_qkv, W_out):
    x = np.ascontiguousarray(np.asarray(x, dtype=np.float32))
    W_qkv = np.ascontiguousarray(np.asarray(W_qkv, dtype=np.float32))
    W_out = np.ascontiguousarray(np.asarray(W_out, dtype=np.float32))
    xT = [np.ascontiguousarray(x[b].T) for b in range(B)]
    in_maps = []
    for c in range(NCORES):
        b, g = c // 4, c % 4
        rq = W_qkv[g * 256:(g + 1) * 256]            # q rows, heads 4g..4g+3
        rk = W_qkv[C + g * 256:C + (g + 1) * 256]    # k rows
        rv = W_qkv[2 * C + g * 256:2 * C + (g + 1) * 256]  # v rows
        wqkvT = np.ascontiguousarray(np.concatenate([rq, rk, rv], axis=0).T)
        woutT = np.ascontiguousarray(W_out[:, g * 256:(g + 1) * 256].T)
        in_maps.append({"xT": xT[b], "wqkvT": wqkvT, "woutT": woutT})
    return in_maps


def kernel(x, W_qkv, W_out):
    nc = _get_nc()
    in_maps = make_in_maps(x, W_qkv, W_out)
    res = run_bass_kernel_spmd(nc, in_maps, core_ids=list(range(NCORES)))
    kernel.last_results = res
    y = np.zeros((B, T, C), dtype=np.float32)
    for c in range(NCORES):
        y[c // 4] += res.results[c]["y"]
    return y



# revision 11
# speedup vs baseline: 1.5362x; 1.5362x over previous
"""Causal self-attention (B=2, T=2048, D=1024, H=16) on 8 trn2 cores.

Sharding: tensor-parallel over heads x data-parallel over batch.
Core c handles batch b = c // 4, head group g = c % 4 (heads 4g..4g+3).
Host pre-slices/pre-transposes weight+activation shards (cast to bf16);
each core returns a partial y (its heads' contribution); host sums
groups of 4.

Kernel structure (all matmuls bf16, psum f32):
  per t-tile tt (512 wide):
    A(tt): qkv projection for that t-slice (ci-outer so input DMA is
           consumed as it arrives)
    outproj(tt-1): output projection of the previous tile (deferred one
           tile so its dependency on the softmax normalization is long
           resolved -> no tensor stall)
    B(tt, pr) for each head pair: attention with a depth-1 software
           pipeline (QK(ss) issued before PV(ss-1)) so TensorE never
           waits on the Scalar-engine exp; both heads of the pair share
           one [128,2,512] psum quad -> one exp instruction per s-chunk.
    norm(tt, pr): 1/L via DVE reciprocal + gpsimd partition_broadcast +
           DVE multiply (no TensorE involvement).
  Causal masking: s-chunks beyond the diagonal are skipped entirely;
  diagonal chunks stream only the t >= s columns (partial-width matmuls)
  plus an affine_select for the 128-wide triangle.
"""

import os
import sys

for _p in ("/opt/trn_rl_repo", "/root/.axon_site/_ro/trn_rl_repo"):
    if os.path.isdir(_p) and _p not in sys.path:
        sys.path.insert(0, _p)

import ml_dtypes
import numpy as np

import concourse.bass as bass
import concourse.mybir as mybir
import concourse.tile as tile
from concourse import bacc
from concourse.bass_utils import run_bass_kernel_spmd

F32 = mybir.dt.float32
BF16 = mybir.dt.bfloat16
U16 = mybir.dt.uint16

B, T, C = 2, 2048, 1024
NHEAD_TOT = 16
DH = 64
NCORES = 8
NH = 4          # heads per core
NPAIR = 2       # head pairs per core
CK = C // 128   # contraction chunks (8)
TT = 512        # t-tile width
NTT = T // TT   # 4
FQK = 2 * NH * DH  # 512 cols of qkv^T for q+k
FV = NH * DH       # 256 cols for v
ONE_BF16 = 0x3F80


def build_nc():
    nc = bacc.Bacc("TRN2", target_bir_lowering=False, debug=False)

    xT = nc.dram_tensor("xT", [C, T], BF16, kind="ExternalInput")
    wqkvT = nc.dram_tensor("wqkvT", [C, FQK + FV], BF16, kind="ExternalInput")
    woutT = nc.dram_tensor("woutT", [NH * DH, C], BF16, kind="ExternalInput")
    y = nc.dram_tensor("y", [T, C], BF16, kind="ExternalOutput")

    EXP = mybir.ActivationFunctionType.Exp

    with tile.TileContext(nc) as tc:
        with (
            tc.tile_pool(name="const", bufs=1) as const,
            tc.tile_pool(name="ptp", bufs=4) as ptp,
            tc.tile_pool(name="bcp", bufs=4) as bcp,
            tc.tile_pool(name="rcp", bufs=4) as rcp,
            tc.tile_pool(name="yp", bufs=2) as yp,
            tc.tile_pool(name="psS", bufs=2, space="PSUM") as psS,
            tc.tile_pool(name="psV", bufs=4, space="PSUM") as psV,
        ):
            # ---- persistent SBUF ----
            xT_sb = const.tile([128, CK, T], BF16)            # x^T (c-major)
            wqkvT_sb = const.tile([128, CK, FQK + FV], BF16)  # cols [q(4x64)|k(4x64)|v(4x64)]
            woutT_sb = const.tile([128, NPAIR, C], BF16)      # W_out^T rows per head pair
            qk_t = [const.tile([128, 4, TT], BF16, name=f"qk_t{i}") for i in range(NTT)]  # [qp0|qp1|kp0|kp1]
            v_t = [const.tile([128, 4, NH, DH + 1], BF16, name=f"v_t{i}") for i in range(NTT)]  # V + ones col
            o_t = [const.tile([128, NPAIR, TT], BF16, name=f"o_t{i}") for i in range(NTT)]  # normalized O^T

            for tt in range(NTT):
                nc.vector.memset(v_t[tt][:, :, :, DH:DH + 1].bitcast(U16), ONE_BF16)

            # ---- DMAs: first tile's deps first ----
            for ci in range(CK):
                nc.sync.dma_start(wqkvT_sb[:, ci, :], wqkvT[ci * 128:(ci + 1) * 128, :])
                nc.sync.dma_start(xT_sb[:, ci, 0:TT], xT[ci * 128:(ci + 1) * 128, 0:TT])
            for tt in range(1, NTT):
                for ci in range(CK):
                    nc.sync.dma_start(xT_sb[:, ci, tt * TT:(tt + 1) * TT],
                                      xT[ci * 128:(ci + 1) * 128, tt * TT:(tt + 1) * TT])
            for pr in range(NPAIR):
                nc.sync.dma_start(woutT_sb[:, pr, :], woutT[pr * 128:(pr + 1) * 128, :])

            def phase_a(tt):
                """QKV projection for t-tile tt."""
                tsl = slice(tt * TT, (tt + 1) * TT)
                for fq in range(2):  # 0: q pairs, 1: k pairs
                    ps = psS.tile([128, 2, TT], F32, tag="ps", name=f"qk{tt}_{fq}")
                    for f2 in range(2):
                        f = 2 * fq + f2
                        for ci in range(CK):
                            nc.tensor.matmul(
                                ps[:, f2, :],
                                wqkvT_sb[:, ci, f * 128:(f + 1) * 128],
                                xT_sb[:, ci, tsl],
                                start=(ci == 0), stop=(ci == CK - 1),
                            )
                    nc.vector.tensor_copy(qk_t[tt][:, 2 * fq:2 * fq + 2, :], ps)
                for sp in range(2):
                    pvv = [psS.tile([128, FV], F32, tag="ps", name=f"v{tt}_{sp}_{k}")
                           for k in range(2)]
                    for k in range(2):
                        si = tt * 4 + sp * 2 + k
                        for ci in range(CK):
                            nc.tensor.matmul(
                                pvv[k],
                                xT_sb[:, ci, si * 128:(si + 1) * 128],
                                wqkvT_sb[:, ci, FQK:FQK + FV],
                                start=(ci == 0), stop=(ci == CK - 1),
                            )
                    for k in range(2):
                        nc.vector.tensor_copy(
                            v_t[tt][:, sp * 2 + k, :, 0:DH],
                            pvv[k].rearrange("p (h d) -> p h d", h=NH),
                        )

            # ones row for the 1/L partition-broadcast matmuls
            ones1 = const.tile([1, 64], BF16)
            nc.vector.memset(ones1.bitcast(U16), ONE_BF16)

            def phase_b(tt, pr, pending):
                """Attention for (t-tile, head pair), depth-1 pipelined.
                `pending` (deferred norm emitter) is flushed after the first
                QK+exp so its broadcast matmul never stalls the PE queue."""
                n_ss = 4 * (tt + 1)
                pv = [psV.tile([DH + 1, TT], F32, tag="pv", name=f"pv{tt}_{pr}_{hi}")
                      for hi in range(2)]

                def emit_pv(pt, t0, ss):
                    for hi in range(2):
                        nc.tensor.matmul(
                            pv[hi][:, t0:TT],
                            v_t[ss // 4][:, ss % 4, pr * 2 + hi, :],
                            pt[:, hi, t0:TT],
                            start=(ss == 0), stop=(ss == n_ss - 1),
                            skip_group_check=True,
                        )

                prev = None
                for ss in range(n_ss):
                    t0 = max(0, 128 * ss - TT * tt)
                    ps = psS.tile([128, 2, TT], F32, tag="ps", name=f"s{tt}_{pr}_{ss}")
                    for hi in range(2):
                        nc.tensor.matmul(
                            ps[:, hi, t0:TT],
                            qk_t[ss // 4][hi * 64:(hi + 1) * 64, 2 + pr,
                                          (ss % 4) * 128:(ss % 4 + 1) * 128],
                            qk_t[tt][hi * 64:(hi + 1) * 64, pr, t0:TT],
                        )
                    pt = ptp.tile([128, 2, TT], BF16, tag="pt", name=f"pt{tt}_{pr}_{ss}")
                    nc.scalar.activation(pt[:, :, t0:TT], ps[:, :, t0:TT], EXP, scale=0.125)
                    if ss >= 4 * tt:  # diagonal chunk: zero the s > t triangle
                        for hi in range(2):
                            nc.gpsimd.affine_select(
                                out=pt[:, hi, t0:t0 + 128],
                                in_=pt[:, hi, t0:t0 + 128],
                                compare_op=mybir.AluOpType.is_ge,
                                fill=0.0,
                                base=0,
                                channel_multiplier=-1,
                                pattern=[[1, 128]],
                            )
                    if pending is not None and ss == 1:
                        pending()
                        pending = None
                    if prev is not None:
                        emit_pv(*prev)
                    prev = (pt, t0, ss)
                emit_pv(*prev)
                if pending is not None:
                    pending()
                return pv

            def norm(tt, pr, pv):
                """o = pv[0:64] / L, L = pv[64] (row of ones dotted with P)."""
                bq = psS.tile([128, TT], F32, tag="ps", name=f"bq{tt}_{pr}")
                bc = bcp.tile([128, TT], BF16, tag="bc", name=f"bc{tt}_{pr}")
                for hi in range(2):
                    rcr = rcp.tile([1, TT], BF16, tag="rcr", name=f"rc{tt}_{pr}_{hi}")
                    with nc.allow_low_precision("bf16 softmax denominators"):
                        nc.vector.reciprocal(rcr, pv[hi][DH:DH + 1, :])
                    nc.tensor.matmul(bq[hi * 64:(hi + 1) * 64, :], ones1, rcr)
                nc.vector.tensor_copy(bc, bq)
                for hi in range(2):
                    nc.vector.tensor_mul(
                        o_t[tt][hi * 64:(hi + 1) * 64, pr, :],
                        pv[hi][0:DH, :],
                        bc[hi * 64:(hi + 1) * 64, :],
                    )

            def outproj(tt):
                """y[tt-slice] = sum_pr o_t[tt]^T @ woutT."""
                for tq in range(4):
                    ps = psS.tile([128, 2, TT], F32, tag="ps", name=f"y{tt}_{tq}")
                    for ot in range(2):
                        for pr in range(NPAIR):
                            nc.tensor.matmul(
                                ps[:, ot, :],
                                o_t[tt][:, pr, tq * 128:(tq + 1) * 128],
                                woutT_sb[:, pr, ot * TT:(ot + 1) * TT],
                                start=(pr == 0), stop=(pr == NPAIR - 1),
                            )
                    yt = yp.tile([128, 2, TT], BF16, tag="yt", name=f"yt{tt}_{tq}")
                    nc.vector.tensor_copy(yt, ps)
                    row = tt * 4 + tq
                    nc.sync.dma_start(
                        y[row * 128:(row + 1) * 128, :],
                        yt.rearrange("p a b -> p (a b)"),
                    )

            pending = None
            for tt in range(NTT):
                phase_a(tt)
                pv0 = phase_b(tt, 0, pending)  # flushes norm(tt-1, 1)
                if tt > 0:
                    outproj(tt - 1)
                pending = (lambda a=tt, b=pv0: norm(a, 0, b))
                pv1 = phase_b(tt, 1, pending)  # flushes norm(tt, 0)
                pending = (lambda a=tt, b=pv1: norm(a, 1, b))
            pending()
            outproj(NTT - 1)

    nc.compile()
    return nc


_NC_CACHE = None


def _get_nc():
    global _NC_CACHE
    if _NC_CACHE is None:
        _NC_CACHE = build_nc()
    return _NC_CACHE


def make_in_maps(x, W_qkv, W_out):
    bf = ml_dtypes.bfloat16
    x = np.asarray(x, dtype=np.float32)
    W_qkv = np.asarray(W_qkv, dtype=np.float32)
    W_out = np.asarray(W_out, dtype=np.float32)
    xT = [np.ascontiguousarray(x[b].T.astype(bf)) for b in range(B)]
    in_maps = []
    for c in range(NCORES):
        b, g = c // 4, c % 4
        rq = W_qkv[g * 256:(g + 1) * 256]            # q rows, heads 4g..4g+3
        rk = W_qkv[C + g * 256:C + (g + 1) * 256]    # k rows
        rv = W_qkv[2 * C + g * 256:2 * C + (g + 1) * 256]  # v rows
        wqkvT = np.ascontiguousarray(
            np.concatenate([rq, rk, rv], axis=0).T.astype(bf))
        woutT = np.ascontiguousarray(W_out[:, g * 256:(g + 1) * 256].T.astype(bf))
        in_maps.append({"xT": xT[b], "wqkvT": wqkvT, "woutT": woutT})
    return in_maps


def kernel(x, W_qkv, W_out):
    nc = _get_nc()
    in_maps = make_in_maps(x, W_qkv, W_out)
    res = run_bass_kernel_spmd(nc, in_maps, core_ids=list(range(NCORES)))
    kernel.last_results = res
    y = np.zeros((B, T, C), dtype=np.float32)
    for c in range(NCORES):
        y[c // 4] += res.results[c]["y"].astype(np.float32)
    return y
